# revision 8
# baseline (speedup 1.0000x reference)
"""Trainium2 Bass kernel: 15x15 valid cross-correlation over a 4096x4096 f32
image, plus scalar bias. Output: [4082, 4082].

Strategy (v2: 2D phase-interleaved matmul, bf16)
------------------------------------------------
Column-shard the output across 8 NeuronCores (512 output columns each; each
core's input slab carries its own 14-column halo, so no device-side exchange).

The PE matmul cost model charges ap_size(out free) cycles per matmul
regardless of K or M, so throughput is maximized by packing as many output
pixels as possible into the 128 PSUM partitions per streamed column. We
interleave the image 2D: partition k = (tr, gc) with tr = row mod 16,
gc = col mod 8, so ALL 128 partitions are outputs:

    XP[(tr, gc), nr, nc] = x[16*nr + tr, 8*nc + gc]      (host-side restride)
    y[16*nr+tr, 8*nc+gc] = sum_{sr in 0..1, sc in 0..2}
        sum_k W_{sr,sc}[k, (tr,gc)] * XP[k, nr+sr, nc+sc]
    W_{sr,sc}[(tr',gc'), (tr,gc)] = w[16*sr+tr'-tr, 8*sc+gc'-gc]  (in-range)

i.e. 6 PSUM-accumulated matmuls per 16x512 output block instead of the 15
banded matmuls per 114x512 window of v1: 6/128 vs 15/114 cycles per output
pixel (2.5x). Operands are bf16 (1 cycle/row at any ap_size; f32r would be
4 cycles/row below ap_size 256), well within the 2e-2 tolerance. The
interleave and de-interleave are free: host-side numpy. Eight row blocks are
fused per matmul (ap_size 512 f32 = the ISA cap / one PSUM bank), minimizing
the per-matmul Ldweights cost (bf16 matmuls always lower to an explicit
Ldweights+Matmult pair on TRN2; walrus ldw-opt cannot fuse them).
"""

import sys

import numpy as np

sys.path.insert(0, "/opt/trn_rl_repo")

H = W = 4096
KH = KW = 15
OH = OW = H - KH + 1  # 4082
NCORES = 8
COLS_PER_CORE = 512  # output columns computed per core

TR, GC = 16, 8  # row/col interleave phases; TR*GC = 128 partitions
NC_IN = 66  # input col slots per core: 528 cols = 512 + 14 halo + 2 pad
NC_OUT = 64  # output col slots per core: 512 cols
NRB = 257  # row blocks 0..256 (block 256 zeros); 16*257 = 4112 rows
NGRP = 32  # groups of 8 row blocks; 32*8 = 256 output blocks = 4096 rows
BLK = 8  # row blocks fused per matmul (ap_size = 8*64 = 512 f32 = one PSUM bank)
STRIP_GROUPS = 2  # groups per DMA strip
STRIP_BLKS = STRIP_GROUPS * BLK + 1  # 17 blocks (1 halo block shared)
NSTRIPS = NGRP // STRIP_GROUPS  # 16
SHIFTS = [(0, 0), (0, 1), (0, 2), (1, 0), (1, 1), (1, 2)]

PAD_ROWS = TR * NRB  # 4112
PAD_COLS = (NCORES - 1) * COLS_PER_CORE + NC_IN * GC  # 4112


def _build_bass(n_reps=1):
    import concourse.mybir as mybir
    from concourse import bacc
    from concourse.tile import TileContext

    f32 = mybir.dt.float32
    bf16 = mybir.dt.bfloat16

    # Bacc (not raw Bass): its finalize() runs move_matmul_waits_to_ldweights
    # + generate_event_semaphores, which legalize Tile's multi-wait
    # instructions for TRN2's 1-wait-per-instruction limit.
    nc = bacc.Bacc()
    xps = nc.declare_dram_parameter(
        "xps", [NSTRIPS, 128, STRIP_BLKS, NC_IN], bf16, isOutput=False
    )
    Wm = nc.declare_dram_parameter("Wm", [128, 6 * 128], bf16, isOutput=False)
    bcol = nc.declare_dram_parameter("bcol", [128, 1], f32, isOutput=False)
    y = nc.declare_dram_parameter(
        "y", [NGRP, 128, BLK, NC_OUT], bf16, isOutput=True
    )

    with TileContext(nc) as tc:
        with (
            tc.tile_pool(name="const", bufs=1) as cpool,
            tc.tile_pool(name="xstrip", bufs=3) as xpool,
            tc.tile_pool(name="obuf", bufs=4) as opool,
            tc.tile_pool(name="psum", bufs=4, space="PSUM") as ppool,
        ):
            W_sb = cpool.tile([128, 6 * 128], bf16)
            nc.sync.dma_start(W_sb[:], Wm[:, :])
            b_sb = cpool.tile([128, 1], f32)
            nc.sync.dma_start(b_sb[:], bcol[:, :])

            def body():
                for st in range(NSTRIPS):
                    xw = xpool.tile([128, STRIP_BLKS, NC_IN], bf16)
                    nc.sync.dma_start(xw[:], xps[st, :, :, :])
                    for j in range(STRIP_GROUPS):
                        g = st * STRIP_GROUPS + j
                        ps = ppool.tile([128, BLK, NC_OUT], f32)
                        for s, (sr, sc) in enumerate(SHIFTS):
                            nc.tensor.matmul(
                                ps[:, :, :],
                                lhsT=W_sb[:, s * 128 : (s + 1) * 128],
                                rhs=xw[
                                    :,
                                    BLK * j + sr : BLK * j + sr + BLK,
                                    sc : sc + NC_OUT,
                                ],
                                start=(s == 0),
                                stop=(s == len(SHIFTS) - 1),
                            )
                        ob = opool.tile([128, BLK, NC_OUT], bf16)
                        nc.vector.tensor_scalar_add(
                            ob[:, :, :], ps[:, :, :], b_sb[:, :]
                        )
                        nc.sync.dma_start(y[g, :, :, :], ob[:, :, :])

            if n_reps == 1:
                body()
            else:
                # Hardware loop: rep count without program-size blowup (all
                # APs are rep-invariant), for reliable rep-delta timing.
                # 4 bodies per iteration amortize the loop's ~2.8us
                # all-engine reset barrier (staggered_reset breaks the
                # cross-strip pipeline; measured slower).
                unroll = 4 if n_reps % 4 == 0 else 1
                with tc.For_i(0, n_reps // unroll):
                    for _u in range(unroll):
                        body()

    # run_bass_kernel_spmd's axon path serializes nc.m directly without
    # finalizing; Bacc needs finalize() -> compile() to legalize waits and
    # allocate registers before the IR hits walrus.
    nc.finalize()
    return nc


def _host_prep(x, w, b):
    from ml_dtypes import bfloat16

    x = np.asarray(x, dtype=np.float32)
    w = np.asarray(w, dtype=np.float32)
    b = np.asarray(b, dtype=np.float32)

    x_pad = np.zeros((PAD_ROWS, PAD_COLS), np.float32)
    x_pad[:H, :W] = x
    x_bf = x_pad.astype(bfloat16)

    # Stationaries: W_s[(tr',gc'), (tr,gc)] = w[16sr+tr'-tr, 8sc+gc'-gc]
    tp = np.arange(TR)
    gp = np.arange(GC)
    Wm_parts = []
    for sr, sc in SHIFTS:
        di = TR * sr + tp[:, None] - tp[None, :]  # [tr', tr]
        dj = GC * sc + gp[:, None] - gp[None, :]  # [gc', gc]
        mr = (di >= 0) & (di < KH)
        mc = (dj >= 0) & (dj < KW)
        Ws = (
            w[np.clip(di, 0, KH - 1)[:, None, :, None],
              np.clip(dj, 0, KW - 1)[None, :, None, :]]
            * (mr[:, None, :, None] & mc[None, :, None, :])
        )  # [tr', gc', tr, gc]
        Wm_parts.append(Ws.reshape(128, 128))
    Wm_np = np.concatenate(Wm_parts, axis=1).astype(bfloat16)

    bcol_np = np.full((128, 1), b[0], np.float32)

    in_maps = []
    for c in range(NCORES):
        slab = x_bf[:, COLS_PER_CORE * c : COLS_PER_CORE * c + NC_IN * GC]
        # XP[(tr,gc), nr, nc] = slab[16nr+tr, 8nc+gc]
        XP = (
            slab.reshape(NRB, TR, NC_IN, GC)
            .transpose(1, 3, 0, 2)
            .reshape(128, NRB, NC_IN)
        )
        strips = np.stack(
            [XP[:, TR * s : TR * s + STRIP_BLKS, :] for s in range(NSTRIPS)]
        )  # [16, 128, 17, 66]
        in_maps.append(
            {
                "xps": np.ascontiguousarray(strips),
                "Wm": Wm_np,
                "bcol": bcol_np,
            }
        )
    return in_maps


def _enable_ldw_opt():
    """No-op (kept for test.py compatibility).

    walrus --enable-ldw-opt=true rejects standalone InstLdweights, and every
    bf16 matmul lowers to a Ldweights+Matmult pair, so the v1 f32r ldw-opt
    trick cannot apply here.
    """


def _unpack_core(y_il):
    """[NGRP, 128, BLK, 64] bf16 interleaved -> [4096, 512] f32."""
    Y = (
        np.asarray(y_il)
        .reshape(NGRP, TR, GC, BLK, NC_OUT)
        .transpose(0, 3, 1, 4, 2)
        .reshape(NGRP * BLK * TR, NC_OUT * GC)
    )
    return Y.astype(np.float32)


def run(x, w, b, n_reps=1):
    """Build, run on 8 cores, return full output."""
    from concourse.bass_utils import run_bass_kernel_spmd

    _enable_ldw_opt()
    nc = _build_bass(n_reps=n_reps)
    in_maps = _host_prep(x, w, b)
    res = run_bass_kernel_spmd(nc, in_maps, list(range(NCORES)))
    outs = [_unpack_core(res.results[c]["y"]) for c in range(NCORES)]
    full = np.concatenate(outs, axis=1)[:OH, :OW]
    return np.ascontiguousarray(full)


def time_reps(x, w, b, n_reps, n_calls=4):
    """Per-call wall times (s) for an n_reps-body program."""
    import time

    from concourse import bass2jax

    nc = _build_bass(n_reps=n_reps)
    in_maps = _host_prep(x, w, b)
    times = []
    for _ in range(n_calls):
        t0 = time.time()
        bass2jax.run_bass_via_pjrt(nc, in_maps, n_cores=NCORES)
        times.append(time.time() - t0)
    return times


def kernel(x, w, b):
    return run(x, w, b)


# revision 9
# speedup vs baseline: 1.0583x; 1.0583x over previous
"""Trainium2 Bass kernel: 15x15 valid cross-correlation over a 4096x4096 f32
image, plus scalar bias. Output: [4082, 4082].

Strategy (v2: 2D phase-interleaved matmul, bf16)
------------------------------------------------
Column-shard the output across 8 NeuronCores (512 output columns each; each
core's input slab carries its own 14-column halo, so no device-side exchange).

The PE matmul cost model charges ap_size(out free) cycles per matmul
regardless of K or M, so throughput is maximized by packing as many output
pixels as possible into the 128 PSUM partitions per streamed column. We
interleave the image 2D: partition k = (tr, gc) with tr = row mod 16,
gc = col mod 8, so ALL 128 partitions are outputs:

    XP[(tr, gc), nr, nc] = x[16*nr + tr, 8*nc + gc]      (host-side restride)
    y[16*nr+tr, 8*nc+gc] = sum_{sr in 0..1, sc in 0..2}
        sum_k W_{sr,sc}[k, (tr,gc)] * XP[k, nr+sr, nc+sc]
    W_{sr,sc}[(tr',gc'), (tr,gc)] = w[16*sr+tr'-tr, 8*sc+gc'-gc]  (in-range)

i.e. 6 PSUM-accumulated matmuls per 16x512 output block instead of the 15
banded matmuls per 114x512 window of v1: 6/128 vs 15/114 cycles per output
pixel (2.5x). Operands are bf16 (1 cycle/row at any ap_size; f32r would be
4 cycles/row below ap_size 256), well within the 2e-2 tolerance. The
interleave and de-interleave are free: host-side numpy. Eight row blocks are
fused per matmul (ap_size 512 f32 = the ISA cap / one PSUM bank), minimizing
the per-matmul Ldweights cost (bf16 matmuls always lower to an explicit
Ldweights+Matmult pair on TRN2; walrus ldw-opt cannot fuse them).
"""

import sys

import numpy as np

sys.path.insert(0, "/opt/trn_rl_repo")

H = W = 4096
KH = KW = 15
OH = OW = H - KH + 1  # 4082
NCORES = 8
COLS_PER_CORE = 512  # output columns computed per core

TR, GC = 16, 8  # row/col interleave phases; TR*GC = 128 partitions
NC_IN = 66  # input col slots per core: 528 cols = 512 + 14 halo + 2 pad
NC_OUT = 64  # output col slots per core: 512 cols
NRB = 257  # row blocks 0..256 (block 256 zeros); 16*257 = 4112 rows
NGRP = 32  # groups of 8 row blocks; 32*8 = 256 output blocks = 4096 rows
BLK = 8  # row blocks fused per matmul (ap_size = 8*64 = 512 f32 = one PSUM bank)
STRIP_GROUPS = 2  # groups per DMA strip
STRIP_BLKS = STRIP_GROUPS * BLK + 1  # 17 blocks (1 halo block shared)
NSTRIPS = NGRP // STRIP_GROUPS  # 16
SHIFTS = [(0, 0), (0, 1), (0, 2), (1, 0), (1, 1), (1, 2)]

PAD_ROWS = TR * NRB  # 4112
PAD_COLS = (NCORES - 1) * COLS_PER_CORE + NC_IN * GC  # 4112


def _build_bass(n_reps=1):
    import concourse.mybir as mybir
    from concourse import bacc
    from concourse.tile import TileContext

    f32 = mybir.dt.float32
    bf16 = mybir.dt.bfloat16

    # Bacc (not raw Bass): its finalize() runs move_matmul_waits_to_ldweights
    # + generate_event_semaphores, which legalize Tile's multi-wait
    # instructions for TRN2's 1-wait-per-instruction limit.
    nc = bacc.Bacc()
    xps = nc.declare_dram_parameter(
        "xps", [NSTRIPS, 128, STRIP_BLKS, NC_IN], bf16, isOutput=False
    )
    Wm = nc.declare_dram_parameter("Wm", [128, 6 * 128], bf16, isOutput=False)
    bcol = nc.declare_dram_parameter("bcol", [128, 1], f32, isOutput=False)
    y = nc.declare_dram_parameter(
        "y", [NGRP, 128, BLK, NC_OUT], bf16, isOutput=True
    )

    with TileContext(nc) as tc:
        with (
            tc.tile_pool(name="const", bufs=1) as cpool,
            tc.tile_pool(name="xstrip", bufs=4) as xpool,
            tc.tile_pool(name="obuf", bufs=6) as opool,
            tc.tile_pool(name="psum", bufs=8, space="PSUM") as ppool,
        ):
            W_sb = cpool.tile([128, 6 * 128], bf16)
            nc.sync.dma_start(W_sb[:], Wm[:, :])
            b_sb = cpool.tile([128, 1], f32)
            nc.sync.dma_start(b_sb[:], bcol[:, :])

            def body():
                for st in range(NSTRIPS):
                    xw = xpool.tile([128, STRIP_BLKS, NC_IN], bf16)
                    nc.sync.dma_start(xw[:], xps[st, :, :, :])
                    for j in range(STRIP_GROUPS):
                        g = st * STRIP_GROUPS + j
                        ps = ppool.tile([128, BLK, NC_OUT], f32)
                        for s, (sr, sc) in enumerate(SHIFTS):
                            nc.tensor.matmul(
                                ps[:, :, :],
                                lhsT=W_sb[:, s * 128 : (s + 1) * 128],
                                rhs=xw[
                                    :,
                                    BLK * j + sr : BLK * j + sr + BLK,
                                    sc : sc + NC_OUT,
                                ],
                                start=(s == 0),
                                stop=(s == len(SHIFTS) - 1),
                            )
                        ob = opool.tile([128, BLK, NC_OUT], bf16)
                        nc.vector.tensor_scalar_add(
                            ob[:, :, :], ps[:, :, :], b_sb[:, :]
                        )
                        nc.sync.dma_start(y[g, :, :, :], ob[:, :, :])

            if n_reps == 1:
                body()
            else:
                # Hardware loop: rep count without program-size blowup (all
                # APs are rep-invariant), for reliable rep-delta timing.
                # 4 bodies per iteration amortize the loop's ~2.8us
                # all-engine reset barrier (staggered_reset breaks the
                # cross-strip pipeline; measured slower).
                unroll = 4 if n_reps % 4 == 0 else 1
                with tc.For_i(0, n_reps // unroll):
                    for _u in range(unroll):
                        body()

    # run_bass_kernel_spmd's axon path serializes nc.m directly without
    # finalizing; Bacc needs finalize() -> compile() to legalize waits and
    # allocate registers before the IR hits walrus.
    nc.finalize()
    return nc


def _host_prep(x, w, b):
    from ml_dtypes import bfloat16

    x = np.asarray(x, dtype=np.float32)
    w = np.asarray(w, dtype=np.float32)
    b = np.asarray(b, dtype=np.float32)

    x_pad = np.zeros((PAD_ROWS, PAD_COLS), np.float32)
    x_pad[:H, :W] = x
    x_bf = x_pad.astype(bfloat16)

    # Stationaries: W_s[(tr',gc'), (tr,gc)] = w[16sr+tr'-tr, 8sc+gc'-gc]
    tp = np.arange(TR)
    gp = np.arange(GC)
    Wm_parts = []
    for sr, sc in SHIFTS:
        di = TR * sr + tp[:, None] - tp[None, :]  # [tr', tr]
        dj = GC * sc + gp[:, None] - gp[None, :]  # [gc', gc]
        mr = (di >= 0) & (di < KH)
        mc = (dj >= 0) & (dj < KW)
        Ws = (
            w[np.clip(di, 0, KH - 1)[:, None, :, None],
              np.clip(dj, 0, KW - 1)[None, :, None, :]]
            * (mr[:, None, :, None] & mc[None, :, None, :])
        )  # [tr', gc', tr, gc]
        Wm_parts.append(Ws.reshape(128, 128))
    Wm_np = np.concatenate(Wm_parts, axis=1).astype(bfloat16)

    bcol_np = np.full((128, 1), b[0], np.float32)

    in_maps = []
    for c in range(NCORES):
        slab = x_bf[:, COLS_PER_CORE * c : COLS_PER_CORE * c + NC_IN * GC]
        # XP[(tr,gc), nr, nc] = slab[16nr+tr, 8nc+gc]
        XP = (
            slab.reshape(NRB, TR, NC_IN, GC)
            .transpose(1, 3, 0, 2)
            .reshape(128, NRB, NC_IN)
        )
        strips = np.stack(
            [XP[:, TR * s : TR * s + STRIP_BLKS, :] for s in range(NSTRIPS)]
        )  # [16, 128, 17, 66]
        in_maps.append(
            {
                "xps": np.ascontiguousarray(strips),
                "Wm": Wm_np,
                "bcol": bcol_np,
            }
        )
    return in_maps


def _enable_ldw_opt():
    """No-op (kept for test.py compatibility).

    walrus --enable-ldw-opt=true rejects standalone InstLdweights, and every
    bf16 matmul lowers to a Ldweights+Matmult pair, so the v1 f32r ldw-opt
    trick cannot apply here.
    """


def _unpack_core(y_il):
    """[NGRP, 128, BLK, 64] bf16 interleaved -> [4096, 512] f32."""
    Y = (
        np.asarray(y_il)
        .reshape(NGRP, TR, GC, BLK, NC_OUT)
        .transpose(0, 3, 1, 4, 2)
        .reshape(NGRP * BLK * TR, NC_OUT * GC)
    )
    return Y.astype(np.float32)


def run(x, w, b, n_reps=1):
    """Build, run on 8 cores, return full output."""
    from concourse.bass_utils import run_bass_kernel_spmd

    _enable_ldw_opt()
    nc = _build_bass(n_reps=n_reps)
    in_maps = _host_prep(x, w, b)
    res = run_bass_kernel_spmd(nc, in_maps, list(range(NCORES)))
    outs = [_unpack_core(res.results[c]["y"]) for c in range(NCORES)]
    full = np.concatenate(outs, axis=1)[:OH, :OW]
    return np.ascontiguousarray(full)


def time_reps(x, w, b, n_reps, n_calls=4):
    """Per-call wall times (s) for an n_reps-body program."""
    import time

    from concourse import bass2jax

    nc = _build_bass(n_reps=n_reps)
    in_maps = _host_prep(x, w, b)
    times = []
    for _ in range(n_calls):
        t0 = time.time()
        bass2jax.run_bass_via_pjrt(nc, in_maps, n_cores=NCORES)
        times.append(time.time() - t0)
    return times


def kernel(x, w, b):
    return run(x, w, b)
